# revision 1
# baseline (speedup 1.0000x reference)
"""Multi-head attention (b=2, t=2048, k=1024, 16 heads) on 8 TRN2 NeuronCores.

Sharding: batch across 2 groups of 4 cores; within a group, heads are
tensor-parallel (4 heads/core, processed as 2 head-pairs).  Per-core pipeline:
  1. fp32r projections from pre-transposed x/W (host supplies x.T, W.T slices)
  2. attention per (head-pair, q-chunk): the two heads share each PE slot at
     full 128x128 array occupancy (keeps the HAM clock gate warm):
       S slot: S.T_A (rows 0-63) + S.T_B (rows 64-127) -> one psum [128, 1024]
       exp [128, 1024] -> P (bf16)
       O slot: col-tiled bf16 matmuls (tile_position (0,0)/(0,64)) accum O.T
       d slot: ones-matmuls producing softmax denominators replicated 64x
     normalize with vector reciprocal + elementwise multiply
  3. per-head-pair 4-core AllGather assembles all 16 heads' O.T
  4. Wo matmul (rows permuted on host to match gather order) -> y.T slice
"""

import sys

if '/opt/trn_rl_repo' not in sys.path:
    sys.path.insert(0, '/opt/trn_rl_repo')

import ml_dtypes
import numpy as np

B = 2
T = 2048
KD = 1024
NH = 16
HS = 64
NCORES = 8
GROUP = 4                 # cores per batch group
NH_LOC = NH // GROUP      # heads per core
NHP = NH_LOC // 2         # head-pairs per core
TSLICE = T // GROUP       # output tokens per core
HFEAT = NH_LOC * HS       # 256 local head features
NKT = T // 128            # 16 key-token tiles
NKD = KD // 128           # 8 model-feature tiles
NQ4 = T // 512            # 4 query chunks of 512

_CACHE = {}


def _build():
    import concourse.bass as bass
    import concourse.mybir as mybir
    import concourse.tile as tile
    from concourse import bacc

    F32 = mybir.dt.float32
    F32R = mybir.dt.float32r
    BF16 = mybir.dt.bfloat16
    AF = mybir.ActivationFunctionType

    nc = bacc.Bacc("TRN2", target_bir_lowering=False, debug=False,
                   num_devices=NCORES)

    xT = nc.dram_tensor("xT", [KD, T], BF16, kind="ExternalInput")
    wqT = nc.dram_tensor("wqT", [KD, HFEAT], BF16, kind="ExternalInput")
    wkT = nc.dram_tensor("wkT", [KD, HFEAT], BF16, kind="ExternalInput")
    wvT = nc.dram_tensor("wvT", [KD, HFEAT], BF16, kind="ExternalInput")
    woT = nc.dram_tensor("woT", [KD, KD], BF16, kind="ExternalInput")
    yT = nc.dram_tensor("yT", [KD, TSLICE], F32, kind="ExternalOutput")

    rgroups = [list(range(GROUP)), list(range(GROUP, 2 * GROUP))]

    with tile.TileContext(nc) as tc:
        with (
            tc.tile_pool(name="qk", bufs=1) as qk_pool,
            tc.tile_pool(name="vpp", bufs=1) as vp_pool,
            tc.tile_pool(name="pt", bufs=6) as pt_pool,
            tc.tile_pool(name="onorm", bufs=6) as on_pool,
            tc.tile_pool(name="rb", bufs=3) as rb_pool,
            tc.tile_pool(name="dram", bufs=1, space="DRAM") as dram,
        ):
            # rows of qt/kt tile hp: 0-63 = head 2hp, 64-127 = head 2hp+1
            qt = [qk_pool.tile([128, T], BF16, name=f"qt{m}", tag=f"qt{m}")
                  for m in range(2)]
            kt = [qk_pool.tile([128, T], BF16, name=f"kt{m}", tag=f"kt{m}")
                  for m in range(2)]
            # V in bf16 token-major: [tok%128, kt_tile, head, 64]
            vp = vp_pool.tile([128, NKT, NH_LOC, HS], BF16)
            ones = vp_pool.tile([128, HS], BF16)
            nc.vector.memset(ones[:], 1.0)

            agin = [[dram.tile([128, 2 * TSLICE], BF16, name=f"agin{h}_{q}",
                                tag=f"agin{h}_{q}") for q in range(2)]
                    for h in range(NHP)]
            # [4, 256, 1024]: dim0 = 2*qhalf + (row>=256), row-major overall
            agout = [dram.tile([4, 256, 2 * TSLICE], BF16, name=f"agout{h}",
                               tag=f"agout{h}") for h in range(NHP)]

            # ---- phase 1: projections ----
            with (
                tc.tile_pool(name="xw", bufs=1) as xw_pool,
                tc.tile_pool(name="ppsum", bufs=1, space="PSUM") as ppsum,
            ):
                wq = xw_pool.tile([128, NKD, HFEAT], BF16)
                wk = xw_pool.tile([128, NKD, HFEAT], BF16)
                wv = xw_pool.tile([128, NKD, HFEAT], BF16)
                xt = xw_pool.tile([128, NKD, T], BF16)
                for k in range(NKD):
                    for wtile, wdram in ((wq, wqT), (wk, wkT), (wv, wvT)):
                        nc.sync.dma_start(
                            wtile[:, k, :], wdram.ap()[128 * k:128 * (k + 1), :])
                    nc.sync.dma_start(
                        xt[:, k, :], xT.ap()[128 * k:128 * (k + 1), :])

                # Q.T / K.T feature-major [256, T]; k-outer so the PE starts
                # as soon as each xt k-slice lands (8 psum banks accumulate)
                for wtile, dst in ((wq, qt), (wk, kt)):
                    acc = [ppsum.tile([128, 512], F32, name=f"acc{i}",
                                      tag=f"acc{i}") for i in range(8)]
                    for k in range(NKD):
                        for m in range(2):
                            for n in range(4):
                                nc.tensor.matmul(
                                    acc[m * 4 + n][:],
                                    wtile[:, k, 128 * m:128 * (m + 1)],
                                    xt[:, k, 512 * n:512 * (n + 1)],
                                    start=(k == 0), stop=(k == NKD - 1),
                                )
                    for m in range(2):
                        for n in range(4):
                            nc.vector.tensor_copy(
                                dst[m][:, 512 * n:512 * (n + 1)],
                                acc[m * 4 + n][:])

                # V token-major -> vp[:, mt, h, :] (bf16)
                for mt in range(NKT):
                    ps = ppsum.tile([128, HFEAT], F32, tag=f"acc{mt % 8}")
                    for k in range(NKD):
                        nc.tensor.matmul(
                            ps[:],
                            xt[:, k, 128 * mt:128 * (mt + 1)],
                            wv[:, k, :],
                            start=(k == 0), stop=(k == NKD - 1),
                        )
                    vcopy = nc.vector.tensor_copy(
                        vp[:, mt, :, :],
                        ps[:].rearrange("p (h d) -> p h d", h=NH_LOC),
                    )

            # ---- phase 2 + 3 share the wo weights ----
            with tc.tile_pool(name="wo", bufs=1) as wo_pool:
                wo = wo_pool.tile([128, NKD, KD], BF16)
                for k in range(NKD):
                    wdma = nc.sync.dma_start(
                        wo[:, k, :], woT.ap()[128 * k:128 * (k + 1), :])
                    tile.add_dep_helper(vcopy.ins, wdma.ins, sync=False,
                                        reason="defer wo prefetch past proj")

                # ---- phase 2: attention, two heads per PE slot ----
                with (
                    tc.tile_pool(name="spsum", bufs=2, space="PSUM") as spsum,
                    tc.tile_pool(name="opsum", bufs=2, space="PSUM") as opsum,
                ):
                    # trailing od/normalize queue: the last od of a chunk
                    # is emitted after the NEXT chunk's first S+exp, so the
                    # in-order PE queue never stalls ACT at chunk boundaries.
                    pending = []

                    def pop_pending():
                        if pending:
                            od, fin = pending.pop(0)
                            od()
                            if fin is not None:
                                fin()

                    for hp in range(NHP):
                        hA, hB = 2 * hp, 2 * hp + 1
                        for q4 in range(NQ4):
                            qs = slice(512 * q4, 512 * (q4 + 1))
                            op = opsum.tile([128, 512], F32, tag="op")
                            dp = opsum.tile([128, 512], F32, tag="dp")
                            pts = []

                            def odslot(km, op=op, dp=dp, pts=pts, hA=hA, hB=hB):
                                nc.tensor.matmul(
                                    op[0:64, :], vp[:, km, hA, :],
                                    pts[km][:, 0:512],
                                    start=(km == 0), stop=(km == NKT - 1),
                                    tile_position=(0, 0))
                                nc.tensor.matmul(
                                    op[64:128, :], vp[:, km, hB, :],
                                    pts[km][:, 512:1024],
                                    start=(km == 0), stop=(km == NKT - 1),
                                    tile_position=(0, 64))
                                nc.tensor.matmul(
                                    dp[0:64, :], ones[:],
                                    pts[km][:, 0:512],
                                    start=(km == 0), stop=(km == NKT - 1),
                                    tile_position=(0, 0))
                                nc.tensor.matmul(
                                    dp[64:128, :], ones[:],
                                    pts[km][:, 512:1024],
                                    start=(km == 0), stop=(km == NKT - 1),
                                    tile_position=(0, 64))

                            def finalize(op=op, dp=dp, hp=hp, q4=q4, qs=qs):
                                rb = rb_pool.tile([128, 512], F32, tag="rb")
                                nc.vector.reciprocal(rb[:], dp[:])
                                onorm = on_pool.tile([128, 512], BF16, tag="on")
                                nc.vector.tensor_mul(onorm[:], op[:], rb[:])
                                nc.sync.dma_start(
                                    agin[hp][q4 // 2][:, (q4 % 2) * 512:
                                                      (q4 % 2) * 512 + 512],
                                    onorm[:])
                                if q4 % 2 == 1:
                                    qh = q4 // 2
                                    nc.gpsimd.collective_compute(
                                        "AllGather",
                                        mybir.AluOpType.bypass,
                                        replica_groups=rgroups,
                                        ins=[agin[hp][qh].opt()],
                                        outs=[agout[hp][2 * qh:2 * qh + 2,
                                                        :, :].opt()],
                                    )

                            for ktile in range(NKT):
                                ks = slice(128 * ktile, 128 * (ktile + 1))
                                sp = spsum.tile([128, 1024], F32, tag="sp")
                                nc.tensor.matmul(
                                    sp[:, 0:512], kt[hp][0:64, ks],
                                    qt[hp][0:64, qs], start=True, stop=True)
                                nc.tensor.matmul(
                                    sp[:, 512:1024], kt[hp][64:128, ks],
                                    qt[hp][64:128, qs], start=True, stop=True)
                                pt = pt_pool.tile([128, 1024], BF16, tag="pt")
                                nc.scalar.activation(pt[:], sp[:], AF.Exp,
                                                     scale=0.03125)
                                pts.append(pt)
                                pop_pending()
                                pending.append(
                                    ((lambda km=ktile, f=odslot: f(km)),
                                     finalize if ktile == NKT - 1 else None))
                    pop_pending()

                # ---- phase 3: Wo ----
                with (
                    tc.tile_pool(name="orhs", bufs=1) as orhs_pool,
                    tc.tile_pool(name="yt", bufs=2) as yt_pool,
                    tc.tile_pool(name="ypsum", bufs=1, space="PSUM") as ypsum,
                ):
                    pid = nc.partition_id()
                    rank2 = pid & 2          # = 2*(rank//2): dim-0 base
                    colo = (pid & 1) * 512   # token-column offset in the q-half
                    rhs = [orhs_pool.tile([128, TSLICE], BF16, name=f"rhs{k}",
                                          tag=f"rhs{k}") for k in range(NKD)]
                    yps = [ypsum.tile([128, TSLICE], F32, name=f"yp{m}",
                                      tag=f"yp{m}") for m in range(NKD)]

                    def wo_pass(ks):
                        for k in ks:
                            hp, src = k // GROUP, k % GROUP
                            d0 = rank2 + (1 if src >= 2 else 0)
                            rows = (src % 2) * 128
                            nc.sync.dma_start(
                                rhs[k][:],
                                agout[hp][bass.ds(d0, 1), rows:rows + 128,
                                          bass.ds(colo, TSLICE)].squeeze(0),
                            )
                        for m in range(NKD):
                            for k in ks:
                                nc.tensor.matmul(
                                    yps[m][:], wo[:, k, 128 * m:128 * (m + 1)],
                                    rhs[k][:],
                                    start=(k == 0), stop=(k == NKD - 1),
                                )

                    # pass A: hp0 k-tiles, runs while the hp1 gathers finish
                    wo_pass(range(GROUP))
                    wo_pass(range(GROUP, NKD))
                    for m in range(NKD):
                        yt_s = yt_pool.tile([128, TSLICE], F32, tag="yt")
                        nc.vector.tensor_copy(yt_s[:], yps[m][:])
                        nc.sync.dma_start(yT.ap()[128 * m:128 * (m + 1), :], yt_s[:])

    nc.compile()
    return nc


def _get_nc():
    if "nc" not in _CACHE:
        _CACHE["nc"] = _build()
    return _CACHE["nc"]


def _make_in_maps(x, Wq, Wk, Wv, Wo):
    # Wo rows permuted to match the AllGather assembly order:
    # feature i = (pair hp, source rank s, head-in-pair a, dim d) -> head 4s+2hp+a
    idx = np.arange(KD)
    hp, rem = idx // (GROUP * 128), idx % (GROUP * 128)
    s, r = rem // 128, rem % 128
    a, d = r // HS, r % HS
    perm = (GROUP * s + 2 * hp + a) * HS + d
    woTp = np.ascontiguousarray(Wo.T[perm]).astype(ml_dtypes.bfloat16)

    in_maps = []
    for c in range(NCORES):
        g, r = c // GROUP, c % GROUP
        rows = slice(r * HFEAT, (r + 1) * HFEAT)
        in_maps.append({
            "xT": np.ascontiguousarray(x[g].T).astype(ml_dtypes.bfloat16),
            "wqT": np.ascontiguousarray(Wq[rows].T).astype(ml_dtypes.bfloat16),
            "wkT": np.ascontiguousarray(Wk[rows].T).astype(ml_dtypes.bfloat16),
            "wvT": np.ascontiguousarray(Wv[rows].T).astype(ml_dtypes.bfloat16),
            "woT": woTp,
        })
    return in_maps


def kernel(x, Wq, Wk, Wv, Wo):
    from concourse import bass_utils

    x = np.asarray(x, dtype=np.float32)
    Wq = np.asarray(Wq, dtype=np.float32)
    Wk = np.asarray(Wk, dtype=np.float32)
    Wv = np.asarray(Wv, dtype=np.float32)
    Wo = np.asarray(Wo, dtype=np.float32)

    nc = _get_nc()
    in_maps = _make_in_maps(x, Wq, Wk, Wv, Wo)
    res = bass_utils.run_bass_kernel_spmd(nc, in_maps, core_ids=list(range(NCORES)))

    out = np.empty((B, T, KD), dtype=np.float32)
    for c in range(NCORES):
        g, r = c // GROUP, c % GROUP
        out[g, r * TSLICE:(r + 1) * TSLICE, :] = res.results[c]["yT"].T
    return out



# revision 9
# speedup vs baseline: 1.0187x; 1.0187x over previous
"""Multi-head attention (b=2, t=2048, k=1024, 16 heads) on 8 TRN2 NeuronCores.

Sharding: batch across 2 groups of 4 cores; within a group, heads are
tensor-parallel (4 heads/core, processed as 2 head-pairs).  Per-core pipeline:
  1. projections from pre-transposed x/W (host supplies x.T, W.T slices)
  2. attention per (head-pair, q-chunk):
       S slot: S.T_A (rows 0-63) + S.T_B (rows 64-127) -> one psum [128, 1024]
       exp [128, 1024] -> P (bf16), split between the Scalar engine (table
         exp) and a custom fused DVE op (deg-3 poly on s/128, squared twice)
       od slot: fused [V | ones] weights -> one matmul per head produces
         O.T (rows 0-63) and the softmax denominator (rows 64-127)
     normalize with DVE reciprocal_approx_fast + elementwise multiply
  3. per (head-pair, q-chunk) 4-core AllGather of O.T [128, 512]
  4. per-q-chunk Wo matmuls on this core's 128-token sub-slice, pipelined
     into phase 2 (rows permuted on host to match gather order)
"""

import sys

if '/opt/trn_rl_repo' not in sys.path:
    sys.path.insert(0, '/opt/trn_rl_repo')

import ml_dtypes
import numpy as np

B = 2
T = 2048
KD = 1024
NH = 16
HS = 64
NCORES = 8
GROUP = 4                 # cores per batch group
NH_LOC = NH // GROUP      # heads per core
NHP = NH_LOC // 2         # head-pairs per core
HFEAT = NH_LOC * HS       # 256 local head features
NKT = T // 128            # 16 key-token tiles
NKD = KD // 128           # 8 model-feature tiles
NQ4 = T // 512            # 4 query chunks of 512
TSUB = 128                # tokens per (q-chunk, core) in the Wo phase

# deg-3 relative-minimax fit of exp(y) on y = s/128 in [-80/128, 80/128],
# constant term pinned to 1; exp(s/32) = p(s)^4.  max rel err 4.1e-3
# (measured max |score| is ~75; same-token Q.K correlation fattens the tail).
EXP_C0 = 0.007828080717
EXP_C1 = 3.133003201e-05
EXP_C2 = 7.726011955e-08

# ktiles handled by the DVE exp op (rest go to the Scalar engine)
DVE_KT = frozenset({3, 5, 7, 9, 11, 13})

# chunk processing order: hp-interleaved so each q4's two gathers finish
# close together and the per-q4 Wo passes pipeline into phase 2
CHUNKS = [(0, 0), (0, 1), (1, 0), (1, 1), (0, 2), (1, 2), (0, 3), (1, 3)]
# wo_chunk(q4) emission points: before CHUNKS[idx] (None -> at end)
WO_EMIT = {4: 0, 5: 1, 7: 2}

_CACHE = {}


def _register_exp_op():
    """Register the fused DVE exp-approx op in the dve_ops registry."""
    from concourse import dve_ops
    from concourse.dve_spec import Spec, Src0, C0, C1, C2, One, Bin, AluOp, sq, lower
    from concourse.dve_uop import DveOpSpec

    name = "EXP_SQ4_ANT"
    for o in dve_ops.OPS:
        if o.name == name:
            return o

    _h = Src0 * C2 + C1
    _h = _h * Src0 + C0
    _p = Bin(AluOp.ADD, Bin(AluOp.MULTIPLY, _h, Src0), One)
    body = sq(sq(_p))

    def ref(in0, in1, c0, c1, c2):
        x = in0.astype(np.float32)
        p = ((x * c2 + c1) * x + c0) * x + np.float32(1.0)
        p = p * p
        return (p * p).astype(np.float32)

    spec = Spec(body=body, reference=ref)
    row = 1 + len(dve_ops.OPS)
    assert row < 0x20
    shas = {}
    for ver in ("v3", "v4"):
        try:
            uops = lower(spec, ver=ver)
            shas[ver] = DveOpSpec(
                name=name, opcode=row, uops=uops, rd1_en=False
            ).sha(ver)
        except Exception:
            pass
    op = dve_ops.DveOp(name, spec, subdim=False, uops_sha=shas)
    dve_ops.OPS.append(op)
    dve_ops._SUB_OPCODE_FOR_NAME[name] = row
    dve_ops.CUSTOM_DVE_SPECS[name] = spec
    return op


def _build():
    import concourse.bass as bass
    import concourse.mybir as mybir
    import concourse.tile as tile
    from concourse import bacc

    exp_op = _register_exp_op()

    F32 = mybir.dt.float32
    BF16 = mybir.dt.bfloat16
    AF = mybir.ActivationFunctionType

    nc = bacc.Bacc("TRN2", target_bir_lowering=False, debug=False,
                   num_devices=NCORES)

    xT = nc.dram_tensor("xT", [KD, T], BF16, kind="ExternalInput")
    wqT = nc.dram_tensor("wqT", [KD, HFEAT], BF16, kind="ExternalInput")
    wkT = nc.dram_tensor("wkT", [KD, HFEAT], BF16, kind="ExternalInput")
    wvT = nc.dram_tensor("wvT", [KD, HFEAT], BF16, kind="ExternalInput")
    woT = nc.dram_tensor("woT", [KD, KD], BF16, kind="ExternalInput")
    yT = nc.dram_tensor("yT", [KD, NQ4, TSUB], F32, kind="ExternalOutput")

    rgroups = [list(range(GROUP)), list(range(GROUP, 2 * GROUP))]

    with tile.TileContext(nc) as tc:
        with (
            tc.tile_pool(name="qk", bufs=1) as qk_pool,
            tc.tile_pool(name="vpp", bufs=1) as vp_pool,
            tc.tile_pool(name="pt", bufs=5) as pt_pool,
            tc.tile_pool(name="onorm", bufs=4) as on_pool,
            tc.tile_pool(name="rb", bufs=2) as rb_pool,
            tc.tile_pool(name="dram", bufs=1, space="DRAM") as dram,
        ):
            # rows of qt/kt tile hp: 0-63 = head 2hp, 64-127 = head 2hp+1
            qt = [qk_pool.tile([128, T], BF16, name=f"qt{m}", tag=f"qt{m}")
                  for m in range(2)]
            kt = [qk_pool.tile([128, T], BF16, name=f"kt{m}", tag=f"kt{m}")
                  for m in range(2)]
            # [ones | V] weights, token-major: [tok%128, kt_tile, head, 128]
            # cols 0-63 = 1.0 (softmax denominator rows land at psum base 0 —
            # custom-DVE ops ignore a nonzero source partition base, so the
            # reciprocal must read base 0), cols 64-127 = V dims
            vpo = vp_pool.tile([128, NKT, NH_LOC, 128], BF16)
            nc.vector.memset(vpo[:, :, :, 0:64], 1.0)

            agin = [[dram.tile([128, 512], BF16, name=f"agin{h}_{q}",
                               tag=f"agin{h}_{q}") for q in range(NQ4)]
                    for h in range(NHP)]
            agout = [[dram.tile([GROUP, 128, 512], BF16, name=f"agout{h}_{q}",
                                tag=f"agout{h}_{q}") for q in range(NQ4)]
                     for h in range(NHP)]

            # ---- phase 1: projections ----
            with (
                tc.tile_pool(name="xw", bufs=1) as xw_pool,
                tc.tile_pool(name="ppsum", bufs=1, space="PSUM") as ppsum,
            ):
                wq = xw_pool.tile([128, NKD, HFEAT], BF16)
                wk = xw_pool.tile([128, NKD, HFEAT], BF16)
                wv = xw_pool.tile([128, NKD, HFEAT], BF16)
                xt = xw_pool.tile([128, NKD, T], BF16)
                for k in range(NKD):
                    for wtile, wdram in ((wq, wqT), (wk, wkT), (wv, wvT)):
                        nc.sync.dma_start(
                            wtile[:, k, :], wdram.ap()[128 * k:128 * (k + 1), :])
                    nc.sync.dma_start(
                        xt[:, k, :], xT.ap()[128 * k:128 * (k + 1), :])

                # Q.T / K.T feature-major [256, T]; k-outer so the PE starts
                # as soon as each xt k-slice lands (8 psum banks accumulate)
                copy_eng = [nc.vector, nc.scalar]
                ncopy = 0
                for wtile, dst in ((wq, qt), (wk, kt)):
                    acc = [ppsum.tile([128, 512], F32, name=f"acc{i}",
                                      tag=f"acc{i}") for i in range(8)]
                    for k in range(NKD):
                        for m in range(2):
                            for n in range(4):
                                nc.tensor.matmul(
                                    acc[m * 4 + n][:],
                                    wtile[:, k, 128 * m:128 * (m + 1)],
                                    xt[:, k, 512 * n:512 * (n + 1)],
                                    start=(k == 0), stop=(k == NKD - 1),
                                )
                    for m in range(2):
                        for n in range(4):
                            eng = copy_eng[ncopy % 2]
                            ncopy += 1
                            if eng is nc.vector:
                                eng.tensor_copy(
                                    dst[m][:, 512 * n:512 * (n + 1)],
                                    acc[m * 4 + n][:])
                            else:
                                eng.copy(
                                    dst[m][:, 512 * n:512 * (n + 1)],
                                    acc[m * 4 + n][:])

                # V token-major -> vpo[:, mt, h, 0:64] (bf16)
                vcopy = None
                for mt in range(NKT):
                    ps = ppsum.tile([128, HFEAT], F32, tag=f"acc{mt % 8}")
                    for k in range(NKD):
                        nc.tensor.matmul(
                            ps[:],
                            xt[:, k, 128 * mt:128 * (mt + 1)],
                            wv[:, k, :],
                            start=(k == 0), stop=(k == NKD - 1),
                        )
                    src = ps[:].rearrange("p (h d) -> p h d", h=NH_LOC)
                    if mt % 2 == 0:
                        vcopy = nc.vector.tensor_copy(
                            vpo[:, mt, :, 64:128], src)
                    else:
                        vcopy = nc.scalar.copy(vpo[:, mt, :, 64:128], src)

            # ---- phase 2 + 3 share the wo weights ----
            with tc.tile_pool(name="wo", bufs=1) as wo_pool:
                wo = wo_pool.tile([128, NKD, KD], BF16)
                for k in range(NKD):
                    wdma = nc.sync.dma_start(
                        wo[:, k, :], woT.ap()[128 * k:128 * (k + 1), :])
                    tile.add_dep_helper(vcopy.ins, wdma.ins, sync=False,
                                        reason="defer wo prefetch past proj")

                with (
                    tc.tile_pool(name="spsum", bufs=2, space="PSUM") as spsum,
                    tc.tile_pool(name="opsum", bufs=1, space="PSUM") as opsum,
                    tc.tile_pool(name="ypsum", bufs=1, space="PSUM") as ypsum,
                    tc.tile_pool(name="orhs", bufs=2) as orhs_pool,
                    tc.tile_pool(name="yt", bufs=4) as yt_pool,
                ):
                    pid = nc.partition_id()
                    colo_sub = (pid & 3) * TSUB
                    rhs = [orhs_pool.tile([128, TSUB], BF16, name=f"rhs{k}",
                                          tag=f"rhs{k}") for k in range(NKD)]
                    # PSUM tiles are bank-granular: pack the 8 m-tiles into
                    # one [128, 8, 128] tile (4 KB/partition = 2 banks)
                    ypk = ypsum.tile([128, NKD, TSUB], F32, name="ypk",
                                     tag="ypk")
                    yps = [ypk[:, m, :] for m in range(NKD)]

                    def wo_chunk(q4):
                        # this core's 128-token sub-slice of q-chunk q4
                        for kidx in range(NKD):
                            hp, s = kidx // GROUP, kidx % GROUP
                            nc.sync.dma_start(
                                rhs[kidx][:],
                                agout[hp][q4][bass.ds(s, 1), :,
                                              bass.ds(colo_sub, TSUB)
                                              ].squeeze(0),
                            )
                        for m in range(NKD):
                            for kidx in range(NKD):
                                nc.tensor.matmul(
                                    yps[m],
                                    wo[:, kidx, 128 * m:128 * (m + 1)],
                                    rhs[kidx][:],
                                    start=(kidx == 0), stop=(kidx == NKD - 1),
                                )
                        for m in range(NKD):
                            yt_s = yt_pool.tile([128, TSUB], F32, tag="yt")
                            if m % 2 == 0:
                                nc.vector.tensor_copy(yt_s[:], yps[m])
                            else:
                                nc.scalar.copy(yt_s[:], yps[m])
                            nc.sync.dma_start(
                                yT.ap()[128 * m:128 * (m + 1), q4, :], yt_s[:])

                    # ---- phase 2: attention ----
                    # trailing od/normalize queue: the last od of a chunk is
                    # emitted after the NEXT chunk's first S+exp, so the
                    # in-order PE queue never stalls exp at chunk boundaries.
                    pending = []

                    def pop_pending():
                        if pending:
                            od, fin = pending.pop(0)
                            od()
                            if fin is not None:
                                fin()

                    for ci, (hp, q4) in enumerate(CHUNKS):
                        if ci in WO_EMIT:
                            pop_pending()
                            wo_chunk(WO_EMIT[ci])
                        hA, hB = 2 * hp, 2 * hp + 1
                        qs = slice(512 * q4, 512 * (q4 + 1))
                        # denominators rows 0-63, O.T rows 64-127;
                        # cols 0-511 head A, cols 512-1023 head B
                        opd = opsum.tile([128, 1024], F32, tag="opd")
                        pts = []

                        def odslot(km, opd=opd, pts=pts, hA=hA, hB=hB):
                            nc.tensor.matmul(
                                opd[:, 0:512], vpo[:, km, hA, :],
                                pts[km][:, 0:512],
                                start=(km == 0), stop=(km == NKT - 1))
                            nc.tensor.matmul(
                                opd[:, 512:1024], vpo[:, km, hB, :],
                                pts[km][:, 512:1024],
                                start=(km == 0), stop=(km == NKT - 1))

                        def finalize(opd=opd, hp=hp, q4=q4):
                            rb = rb_pool.tile([64, 1024], F32, tag="rb")
                            nc.vector.reciprocal_approx_fast(
                                rb[:], opd[0:64, :])
                            onorm = on_pool.tile([64, 1024], BF16, tag="on")
                            nc.vector.tensor_mul(onorm[:], opd[64:128, :], rb[:])
                            nc.sync.dma_start(
                                agin[hp][q4][0:64, :], onorm[:, 0:512])
                            nc.sync.dma_start(
                                agin[hp][q4][64:128, :], onorm[:, 512:1024])
                            nc.gpsimd.collective_compute(
                                "AllGather",
                                mybir.AluOpType.bypass,
                                replica_groups=rgroups,
                                ins=[agin[hp][q4].opt()],
                                outs=[agout[hp][q4].opt()],
                            )

                        for ktile in range(NKT):
                            ks = slice(128 * ktile, 128 * (ktile + 1))
                            sp = spsum.tile([128, 1024], F32, tag="sp")
                            nc.tensor.matmul(
                                sp[:, 0:512], kt[hp][0:64, ks],
                                qt[hp][0:64, qs], start=True, stop=True)
                            nc.tensor.matmul(
                                sp[:, 512:1024], kt[hp][64:128, ks],
                                qt[hp][64:128, qs], start=True, stop=True)
                            pt = pt_pool.tile([128, 1024], BF16, tag="pt")
                            if ktile in DVE_KT:
                                nc.vector._custom_dve(
                                    exp_op, out=pt[:], in0=sp[:],
                                    s0=EXP_C0, s1=EXP_C1, imm2=EXP_C2)
                            else:
                                nc.scalar.activation(pt[:], sp[:], AF.Exp,
                                                     scale=0.03125)
                            pts.append(pt)
                            pop_pending()
                            pending.append(
                                ((lambda km=ktile, f=odslot: f(km)),
                                 finalize if ktile == NKT - 1 else None))
                    pop_pending()
                    wo_chunk(3)

    nc.compile()
    return nc


def _get_nc():
    if "nc" not in _CACHE:
        _CACHE["nc"] = _build()
    return _CACHE["nc"]


def _make_in_maps(x, Wq, Wk, Wv, Wo):
    # Wo rows permuted to match the AllGather assembly order:
    # f = 128*kidx + row, kidx = 4*hp + s  ->  head 4s+2hp+(row>=64), dim row%64
    f = np.arange(KD)
    kidx, row = f // 128, f % 128
    hp, s = kidx // GROUP, kidx % GROUP
    head = GROUP * s + 2 * hp + (row >= HS)
    perm = head * HS + row % HS
    woTp = np.ascontiguousarray(Wo.T[perm]).astype(ml_dtypes.bfloat16)

    in_maps = []
    for c in range(NCORES):
        g, r = c // GROUP, c % GROUP
        rows = slice(r * HFEAT, (r + 1) * HFEAT)
        in_maps.append({
            "xT": np.ascontiguousarray(x[g].T).astype(ml_dtypes.bfloat16),
            "wqT": np.ascontiguousarray(Wq[rows].T).astype(ml_dtypes.bfloat16),
            "wkT": np.ascontiguousarray(Wk[rows].T).astype(ml_dtypes.bfloat16),
            "wvT": np.ascontiguousarray(Wv[rows].T).astype(ml_dtypes.bfloat16),
            "woT": woTp,
        })
    return in_maps


def kernel(x, Wq, Wk, Wv, Wo):
    from concourse import bass_utils

    x = np.asarray(x, dtype=np.float32)
    Wq = np.asarray(Wq, dtype=np.float32)
    Wk = np.asarray(Wk, dtype=np.float32)
    Wv = np.asarray(Wv, dtype=np.float32)
    Wo = np.asarray(Wo, dtype=np.float32)

    nc = _get_nc()
    in_maps = _make_in_maps(x, Wq, Wk, Wv, Wo)
    res = bass_utils.run_bass_kernel_spmd(nc, in_maps, core_ids=list(range(NCORES)))

    out = np.empty((B, T, KD), dtype=np.float32)
    for c in range(NCORES):
        g, r = c // GROUP, c % GROUP
        yTc = res.results[c]["yT"]          # [KD, NQ4, TSUB]
        for q4 in range(NQ4):
            out[g, 512 * q4 + TSUB * r: 512 * q4 + TSUB * (r + 1), :] = \
                yTc[:, q4, :].T
    return out


# revision 13
# speedup vs baseline: 1.1096x; 1.0892x over previous
"""Multi-head attention (b=2, t=2048, k=1024, 16 heads) on 8 TRN2 NeuronCores.

Sharding: batch across 2 groups of 4 cores; within a group, heads are
tensor-parallel (4 heads/core, processed as 2 head-pairs).  Per-core pipeline:
  1. projections from pre-transposed x/W (host supplies x.T, W.T slices)
  2. attention per (head-pair, q-chunk):
       S slot: S.T_A (rows 0-63) + S.T_B (rows 64-127) -> one psum [128, 1024]
       exp [128, 1024] -> P (bf16), split between the Scalar engine (table
         exp) and a custom fused DVE op (deg-3 poly on s/128, squared twice)
       od slot: fused [V | ones] weights -> one matmul per head produces
         O.T (rows 0-63) and the softmax denominator (rows 64-127)
     normalize with DVE reciprocal_approx_fast + elementwise multiply
  3. per (head-pair, q-chunk) 4-core AllGather of O.T [128, 512]
  4. per-q-chunk Wo matmuls on this core's 128-token sub-slice, pipelined
     into phase 2 (rows permuted on host to match gather order)
"""

import sys

if '/opt/trn_rl_repo' not in sys.path:
    sys.path.insert(0, '/opt/trn_rl_repo')

import ml_dtypes
import numpy as np

B = 2
T = 2048
KD = 1024
NH = 16
HS = 64
NCORES = 8
GROUP = 4                 # cores per batch group
NH_LOC = NH // GROUP      # heads per core
NHP = NH_LOC // 2         # head-pairs per core
HFEAT = NH_LOC * HS       # 256 local head features
NKT = T // 128            # 16 key-token tiles
NKD = KD // 128           # 8 model-feature tiles
NQ4 = T // 512            # 4 query chunks of 512
TSUB = 128                # tokens per (q-chunk, core) in the Wo phase

# deg-3 relative-minimax fit of exp(y) on y = s/128 in [-80/128, 80/128],
# constant term pinned to 1; exp(s/32) = p(s)^4.  max rel err 4.1e-3
# (measured max |score| is ~75; same-token Q.K correlation fattens the tail).
EXP_C0 = 0.007828080717
EXP_C1 = 3.133003201e-05
EXP_C2 = 7.726011955e-08

# ktiles handled by the DVE exp op (rest go to the Scalar engine)
DVE_KT = frozenset({3, 5, 7, 9, 11, 13})

# chunk processing order: hp-interleaved so each q4's two gathers finish
# close together and the per-q4 Wo passes pipeline into phase 2
CHUNKS = [(0, 0), (0, 1), (1, 0), (1, 1), (0, 2), (1, 2), (0, 3), (1, 3)]
# wo_chunk(q4) emission points: before CHUNKS[idx] (None -> at end)
WO_EMIT = {4: 0, 5: 1, 7: 2}

_CACHE = {}


def _register_exp_op():
    """Register the fused DVE exp-approx op in the dve_ops registry."""
    from concourse import dve_ops
    from concourse.dve_spec import Spec, Src0, C0, C1, C2, One, Bin, AluOp, sq, lower
    from concourse.dve_uop import DveOpSpec

    name = "EXP_SQ4_ANT"
    for o in dve_ops.OPS:
        if o.name == name:
            return o

    _h = Src0 * C2 + C1
    _h = _h * Src0 + C0
    _p = Bin(AluOp.ADD, Bin(AluOp.MULTIPLY, _h, Src0), One)
    body = sq(sq(_p))

    def ref(in0, in1, c0, c1, c2):
        x = in0.astype(np.float32)
        p = ((x * c2 + c1) * x + c0) * x + np.float32(1.0)
        p = p * p
        return (p * p).astype(np.float32)

    spec = Spec(body=body, reference=ref)
    row = 1 + len(dve_ops.OPS)
    assert row < 0x20
    shas = {}
    for ver in ("v3", "v4"):
        try:
            uops = lower(spec, ver=ver)
            shas[ver] = DveOpSpec(
                name=name, opcode=row, uops=uops, rd1_en=False
            ).sha(ver)
        except Exception:
            pass
    op = dve_ops.DveOp(name, spec, subdim=False, uops_sha=shas)
    dve_ops.OPS.append(op)
    dve_ops._SUB_OPCODE_FOR_NAME[name] = row
    dve_ops.CUSTOM_DVE_SPECS[name] = spec
    return op


def _build():
    import concourse.bass as bass
    import concourse.mybir as mybir
    import concourse.tile as tile
    from concourse import bacc

    exp_op = _register_exp_op()

    F32 = mybir.dt.float32
    BF16 = mybir.dt.bfloat16
    AF = mybir.ActivationFunctionType

    nc = bacc.Bacc("TRN2", target_bir_lowering=False, debug=False,
                   num_devices=NCORES)

    xT = nc.dram_tensor("xT", [KD, T], BF16, kind="ExternalInput")
    wqT = nc.dram_tensor("wqT", [KD, HFEAT], BF16, kind="ExternalInput")
    wkT = nc.dram_tensor("wkT", [KD, HFEAT], BF16, kind="ExternalInput")
    wvT = nc.dram_tensor("wvT", [KD, HFEAT], BF16, kind="ExternalInput")
    woT = nc.dram_tensor("woT", [KD, KD], BF16, kind="ExternalInput")
    yT = nc.dram_tensor("yT", [KD, NQ4, TSUB], F32, kind="ExternalOutput")

    rgroups = [list(range(GROUP)), list(range(GROUP, 2 * GROUP))]

    with tile.TileContext(nc) as tc:
        with (
            tc.tile_pool(name="qk", bufs=1) as qk_pool,
            tc.tile_pool(name="vpp", bufs=1) as vp_pool,
            tc.tile_pool(name="pt", bufs=6) as pt_pool,
            tc.tile_pool(name="onorm", bufs=4) as on_pool,
            tc.tile_pool(name="rb", bufs=2) as rb_pool,
            tc.tile_pool(name="dram", bufs=1, space="DRAM") as dram,
        ):
            # rows of qt/kt tile hp: 0-63 = head 2hp, 64-127 = head 2hp+1
            qt = [qk_pool.tile([128, T], BF16, name=f"qt{m}", tag=f"qt{m}")
                  for m in range(2)]
            kt = [qk_pool.tile([128, T], BF16, name=f"kt{m}", tag=f"kt{m}")
                  for m in range(2)]
            # [ones | V] weights, token-major: [tok%128, kt_tile, head, 128]
            # cols 0-63 = 1.0 (softmax denominator rows land at psum base 0 —
            # custom-DVE ops ignore a nonzero source partition base, so the
            # reciprocal must read base 0), cols 64-127 = V dims
            vpo = vp_pool.tile([128, NKT, NH_LOC, 128], BF16)
            nc.vector.memset(vpo[:, :, :, 0:64], 1.0)

            agin = [[dram.tile([128, 512], BF16, name=f"agin{h}_{q}",
                               tag=f"agin{h}_{q}") for q in range(NQ4)]
                    for h in range(NHP)]
            agout = [[dram.tile([GROUP, 128, 512], BF16, name=f"agout{h}_{q}",
                                tag=f"agout{h}_{q}") for q in range(NQ4)]
                     for h in range(NHP)]

            # ---- phase 1: projections ----
            with (
                tc.tile_pool(name="xw", bufs=1) as xw_pool,
                tc.tile_pool(name="ppsum", bufs=1, space="PSUM") as ppsum,
            ):
                wq = xw_pool.tile([128, NKD, HFEAT], BF16)
                wk = xw_pool.tile([128, NKD, HFEAT], BF16)
                wv = xw_pool.tile([128, NKD, HFEAT], BF16)
                xt = xw_pool.tile([128, NKD, T], BF16)
                for k in range(NKD):
                    for wtile, wdram in ((wq, wqT), (wk, wkT), (wv, wvT)):
                        nc.sync.dma_start(
                            wtile[:, k, :], wdram.ap()[128 * k:128 * (k + 1), :])
                    nc.sync.dma_start(
                        xt[:, k, :], xT.ap()[128 * k:128 * (k + 1), :])

                # Q.T / K.T feature-major [256, T]; k-inner accumulation
                # chains hide LDWEIGHTS under the previous matmul (measured
                # 131 vs 260 ns/MM).  hp-0 halves first so attention can
                # start while hp-1 projects.
                ncopy = 0
                nacc = 0
                for m in range(2):
                    for wtile, dst in ((wk, kt), (wq, qt)):
                        for n in range(4):
                            acc = ppsum.tile([128, 512], F32,
                                             tag=f"acc{nacc % 8}")
                            nacc += 1
                            for k in range(NKD):
                                nc.tensor.matmul(
                                    acc[:],
                                    wtile[:, k, 128 * m:128 * (m + 1)],
                                    xt[:, k, 512 * n:512 * (n + 1)],
                                    start=(k == 0), stop=(k == NKD - 1),
                                )
                            ncopy += 1
                            if ncopy % 2 == 0:
                                nc.vector.tensor_copy(
                                    dst[m][:, 512 * n:512 * (n + 1)], acc[:])
                            else:
                                nc.scalar.copy(
                                    dst[m][:, 512 * n:512 * (n + 1)], acc[:])

                # V token-major -> vpo[:, mt, h, 0:64] (bf16)
                vcopy = None
                for mt in range(NKT):
                    ps = ppsum.tile([128, HFEAT], F32, tag=f"acc{mt % 8}")
                    for k in range(NKD):
                        nc.tensor.matmul(
                            ps[:],
                            xt[:, k, 128 * mt:128 * (mt + 1)],
                            wv[:, k, :],
                            start=(k == 0), stop=(k == NKD - 1),
                        )
                    src = ps[:].rearrange("p (h d) -> p h d", h=NH_LOC)
                    if mt % 2 == 0:
                        vcopy = nc.vector.tensor_copy(
                            vpo[:, mt, :, 64:128], src)
                    else:
                        vcopy = nc.scalar.copy(vpo[:, mt, :, 64:128], src)

            # ---- phase 2 + 3 share the wo weights ----
            with tc.tile_pool(name="wo", bufs=1) as wo_pool:
                wo = wo_pool.tile([128, NKD, KD], BF16)
                for k in range(NKD):
                    wdma = nc.sync.dma_start(
                        wo[:, k, :], woT.ap()[128 * k:128 * (k + 1), :])
                    tile.add_dep_helper(vcopy.ins, wdma.ins, sync=False,
                                        reason="defer wo prefetch past proj")

                with (
                    tc.tile_pool(name="spsum", bufs=2, space="PSUM") as spsum,
                    tc.tile_pool(name="opsum", bufs=1, space="PSUM") as opsum,
                    tc.tile_pool(name="ypsum", bufs=1, space="PSUM") as ypsum,
                    tc.tile_pool(name="orhs", bufs=2) as orhs_pool,
                    tc.tile_pool(name="yt", bufs=4) as yt_pool,
                ):
                    pid = nc.partition_id()
                    colo_sub = (pid & 3) * TSUB
                    rhs = [orhs_pool.tile([128, TSUB], BF16, name=f"rhs{k}",
                                          tag=f"rhs{k}") for k in range(NKD)]
                    # PSUM tiles are bank-granular: pack the 8 m-tiles into
                    # one [128, 8, 128] tile (4 KB/partition = 2 banks)
                    ypk = ypsum.tile([128, NKD, TSUB], F32, name="ypk",
                                     tag="ypk")
                    yps = [ypk[:, m, :] for m in range(NKD)]

                    def wo_chunk(q4):
                        # this core's 128-token sub-slice of q-chunk q4
                        for kidx in range(NKD):
                            hp, s = kidx // GROUP, kidx % GROUP
                            nc.sync.dma_start(
                                rhs[kidx][:],
                                agout[hp][q4][bass.ds(s, 1), :,
                                              bass.ds(colo_sub, TSUB)
                                              ].squeeze(0),
                            )
                        for m in range(NKD):
                            for kidx in range(NKD):
                                nc.tensor.matmul(
                                    yps[m],
                                    wo[:, kidx, 128 * m:128 * (m + 1)],
                                    rhs[kidx][:],
                                    start=(kidx == 0), stop=(kidx == NKD - 1),
                                )
                        for m in range(NKD):
                            yt_s = yt_pool.tile([128, TSUB], F32, tag="yt")
                            if m % 2 == 0:
                                nc.vector.tensor_copy(yt_s[:], yps[m])
                            else:
                                nc.scalar.copy(yt_s[:], yps[m])
                            nc.sync.dma_start(
                                yT.ap()[128 * m:128 * (m + 1), q4, :], yt_s[:])

                    # ---- phase 2: attention ----
                    # trailing od/normalize queue: od(k) is emitted two
                    # ktiles after exp(k) is issued, so the in-order PE queue
                    # never waits on the ~1.2us exp latency.
                    pending = []

                    def pop_pending(depth=2):
                        while len(pending) >= depth:
                            od, fin = pending.pop(0)
                            od()
                            if fin is not None:
                                fin()

                    for ci, (hp, q4) in enumerate(CHUNKS):
                        if ci in WO_EMIT:
                            pop_pending(1)
                            wo_chunk(WO_EMIT[ci])
                        hA, hB = 2 * hp, 2 * hp + 1
                        qs = slice(512 * q4, 512 * (q4 + 1))
                        # denominators rows 0-63, O.T rows 64-127;
                        # cols 0-511 head A, cols 512-1023 head B
                        opd = opsum.tile([128, 1024], F32, tag="opd")
                        pts = []

                        def odslot(km, opd=opd, pts=pts, hA=hA, hB=hB):
                            nc.tensor.matmul(
                                opd[:, 0:512], vpo[:, km, hA, :],
                                pts[km][:, 0:512],
                                start=(km == 0), stop=(km == NKT - 1))
                            nc.tensor.matmul(
                                opd[:, 512:1024], vpo[:, km, hB, :],
                                pts[km][:, 512:1024],
                                start=(km == 0), stop=(km == NKT - 1))

                        def finalize(opd=opd, hp=hp, q4=q4):
                            rb = rb_pool.tile([64, 1024], F32, tag="rb")
                            nc.vector.reciprocal_approx_fast(
                                rb[:], opd[0:64, :])
                            onorm = on_pool.tile([64, 1024], BF16, tag="on")
                            nc.vector.tensor_mul(onorm[:], opd[64:128, :], rb[:])
                            nc.sync.dma_start(
                                agin[hp][q4][0:64, :], onorm[:, 0:512])
                            nc.sync.dma_start(
                                agin[hp][q4][64:128, :], onorm[:, 512:1024])
                            nc.gpsimd.collective_compute(
                                "AllGather",
                                mybir.AluOpType.bypass,
                                replica_groups=rgroups,
                                ins=[agin[hp][q4].opt()],
                                outs=[agout[hp][q4].opt()],
                            )

                        for ktile in range(NKT):
                            ks = slice(128 * ktile, 128 * (ktile + 1))
                            sp = spsum.tile([128, 1024], F32, tag="sp")
                            nc.tensor.matmul(
                                sp[:, 0:512], kt[hp][0:64, ks],
                                qt[hp][0:64, qs], start=True, stop=True)
                            nc.tensor.matmul(
                                sp[:, 512:1024], kt[hp][64:128, ks],
                                qt[hp][64:128, qs], start=True, stop=True)
                            pt = pt_pool.tile([128, 1024], BF16, tag="pt")
                            if ktile in DVE_KT:
                                nc.vector._custom_dve(
                                    exp_op, out=pt[:], in0=sp[:],
                                    s0=EXP_C0, s1=EXP_C1, imm2=EXP_C2)
                            else:
                                nc.scalar.activation(pt[:], sp[:], AF.Exp,
                                                     scale=0.03125)
                            pts.append(pt)
                            pop_pending(3)
                            pending.append(
                                ((lambda km=ktile, f=odslot: f(km)),
                                 finalize if ktile == NKT - 1 else None))
                    pop_pending(1)
                    wo_chunk(3)

    nc.compile()
    return nc


def _get_nc():
    if "nc" not in _CACHE:
        _CACHE["nc"] = _build()
    return _CACHE["nc"]


def _make_in_maps(x, Wq, Wk, Wv, Wo):
    # Wo rows permuted to match the AllGather assembly order:
    # f = 128*kidx + row, kidx = 4*hp + s  ->  head 4s+2hp+(row>=64), dim row%64
    f = np.arange(KD)
    kidx, row = f // 128, f % 128
    hp, s = kidx // GROUP, kidx % GROUP
    head = GROUP * s + 2 * hp + (row >= HS)
    perm = head * HS + row % HS
    woTp = np.ascontiguousarray(Wo.T[perm]).astype(ml_dtypes.bfloat16)

    in_maps = []
    for c in range(NCORES):
        g, r = c // GROUP, c % GROUP
        rows = slice(r * HFEAT, (r + 1) * HFEAT)
        in_maps.append({
            "xT": np.ascontiguousarray(x[g].T).astype(ml_dtypes.bfloat16),
            "wqT": np.ascontiguousarray(Wq[rows].T).astype(ml_dtypes.bfloat16),
            "wkT": np.ascontiguousarray(Wk[rows].T).astype(ml_dtypes.bfloat16),
            "wvT": np.ascontiguousarray(Wv[rows].T).astype(ml_dtypes.bfloat16),
            "woT": woTp,
        })
    return in_maps


def kernel(x, Wq, Wk, Wv, Wo):
    from concourse import bass_utils

    x = np.asarray(x, dtype=np.float32)
    Wq = np.asarray(Wq, dtype=np.float32)
    Wk = np.asarray(Wk, dtype=np.float32)
    Wv = np.asarray(Wv, dtype=np.float32)
    Wo = np.asarray(Wo, dtype=np.float32)

    nc = _get_nc()
    in_maps = _make_in_maps(x, Wq, Wk, Wv, Wo)
    res = bass_utils.run_bass_kernel_spmd(nc, in_maps, core_ids=list(range(NCORES)))

    out = np.empty((B, T, KD), dtype=np.float32)
    for c in range(NCORES):
        g, r = c // GROUP, c % GROUP
        yTc = res.results[c]["yT"]          # [KD, NQ4, TSUB]
        for q4 in range(NQ4):
            out[g, 512 * q4 + TSUB * r: 512 * q4 + TSUB * (r + 1), :] = \
                yTc[:, q4, :].T
    return out


# revision 21
# speedup vs baseline: 1.1212x; 1.0105x over previous
"""Multi-head attention (b=2, t=2048, k=1024, 16 heads) on 8 TRN2 NeuronCores.

Sharding: batch across 2 groups of 4 cores; within a group, heads are
tensor-parallel (4 heads/core, processed as 2 head-pairs).  Per-core pipeline:
  1. projections from pre-transposed x/W (host supplies x.T, W.T slices)
  2. attention per (head-pair, q-chunk):
       S slot: S.T_A (rows 0-63) + S.T_B (rows 64-127) -> one psum [128, 1024]
       exp [128, 1024] -> P (bf16), split between the Scalar engine (table
         exp) and a custom fused DVE op (deg-3 poly on s/128, squared twice)
       od slot: fused [V | ones] weights -> one matmul per head produces
         O.T (rows 0-63) and the softmax denominator (rows 64-127)
     normalize with DVE reciprocal_approx_fast + elementwise multiply
  3. per (head-pair, q-chunk) 4-core AllGather of O.T [128, 512]
  4. per-q-chunk Wo matmuls on this core's 128-token sub-slice, pipelined
     into phase 2 (rows permuted on host to match gather order)
"""

import sys

if '/opt/trn_rl_repo' not in sys.path:
    sys.path.insert(0, '/opt/trn_rl_repo')

import ml_dtypes
import numpy as np

B = 2
T = 2048
KD = 1024
NH = 16
HS = 64
NCORES = 8
GROUP = 4                 # cores per batch group
NH_LOC = NH // GROUP      # heads per core
NHP = NH_LOC // 2         # head-pairs per core
HFEAT = NH_LOC * HS       # 256 local head features
NKT = T // 128            # 16 key-token tiles
NKD = KD // 128           # 8 model-feature tiles
NQ4 = T // 512            # 4 query chunks of 512
TSUB = 128                # tokens per (q-chunk, core) in the Wo phase

# deg-3 relative-minimax fit of exp(y) on y = s/128 in [-80/128, 80/128],
# constant term pinned to 1; exp(s/32) = p(s)^4.  max rel err 4.1e-3
# (measured max |score| is ~75; same-token Q.K correlation fattens the tail).
EXP_C0 = 0.007828080717
EXP_C1 = 3.133003201e-05
EXP_C2 = 7.726011955e-08

# ktiles handled by the DVE exp op (rest go to the Scalar engine)
DVE_KT = frozenset({3, 5, 7, 9, 11, 13})

# chunk processing order: hp-interleaved so each q4's two gathers finish
# close together and the per-q4 Wo passes pipeline into phase 2
CHUNKS = [(0, 0), (0, 1), (1, 0), (1, 1), (0, 2), (1, 2), (0, 3), (1, 3)]
# wo_chunk(q4) emission points: before CHUNKS[idx] (None -> at end)
WO_EMIT = {4: 0, 5: 1, 7: 2}

_CACHE = {}


def _register_exp_op():
    """Register the fused DVE exp-approx op in the dve_ops registry."""
    from concourse import dve_ops
    from concourse.dve_spec import Spec, Src0, C0, C1, C2, One, Bin, AluOp, sq, lower
    from concourse.dve_uop import DveOpSpec

    name = "EXP_SQ4_ANT"
    for o in dve_ops.OPS:
        if o.name == name:
            return o

    _h = Src0 * C2 + C1
    _h = _h * Src0 + C0
    _p = Bin(AluOp.ADD, Bin(AluOp.MULTIPLY, _h, Src0), One)
    body = sq(sq(_p))

    def ref(in0, in1, c0, c1, c2):
        x = in0.astype(np.float32)
        p = ((x * c2 + c1) * x + c0) * x + np.float32(1.0)
        p = p * p
        return (p * p).astype(np.float32)

    spec = Spec(body=body, reference=ref)
    row = 1 + len(dve_ops.OPS)
    assert row < 0x20
    shas = {}
    for ver in ("v3", "v4"):
        try:
            uops = lower(spec, ver=ver)
            shas[ver] = DveOpSpec(
                name=name, opcode=row, uops=uops, rd1_en=False
            ).sha(ver)
        except Exception:
            pass
    op = dve_ops.DveOp(name, spec, subdim=False, uops_sha=shas)
    dve_ops.OPS.append(op)
    dve_ops._SUB_OPCODE_FOR_NAME[name] = row
    dve_ops.CUSTOM_DVE_SPECS[name] = spec
    return op


def _build():
    import concourse.bass as bass
    import concourse.mybir as mybir
    import concourse.tile as tile
    from concourse import bacc

    exp_op = _register_exp_op()

    F32 = mybir.dt.float32
    BF16 = mybir.dt.bfloat16
    AF = mybir.ActivationFunctionType

    nc = bacc.Bacc("TRN2", target_bir_lowering=False, debug=False,
                   num_devices=NCORES)

    xT = nc.dram_tensor("xT", [KD, T], BF16, kind="ExternalInput")
    wqT = nc.dram_tensor("wqT", [KD, HFEAT], BF16, kind="ExternalInput")
    wkT = nc.dram_tensor("wkT", [KD, HFEAT], BF16, kind="ExternalInput")
    wvT = nc.dram_tensor("wvT", [KD, HFEAT], BF16, kind="ExternalInput")
    woT = nc.dram_tensor("woT", [KD, KD], BF16, kind="ExternalInput")
    yT = nc.dram_tensor("yT", [KD, NQ4, TSUB], F32, kind="ExternalOutput")

    rgroups = [list(range(GROUP)), list(range(GROUP, 2 * GROUP))]

    with tile.TileContext(nc) as tc:
        with (
            tc.tile_pool(name="qk", bufs=1) as qk_pool,
            tc.tile_pool(name="vpp", bufs=1) as vp_pool,
            tc.tile_pool(name="pt", bufs=6) as pt_pool,
            tc.tile_pool(name="onorm", bufs=4) as on_pool,
            tc.tile_pool(name="rb", bufs=2) as rb_pool,
            tc.tile_pool(name="dram", bufs=1, space="DRAM") as dram,
        ):
            # rows of qt/kt tile hp: 0-63 = head 2hp, 64-127 = head 2hp+1
            qt = [qk_pool.tile([128, T], BF16, name=f"qt{m}", tag=f"qt{m}")
                  for m in range(2)]
            kt = [qk_pool.tile([128, T], BF16, name=f"kt{m}", tag=f"kt{m}")
                  for m in range(2)]
            # [ones | V] weights, token-major: [tok%128, kt_tile, head, 128]
            # cols 0-63 = 1.0 (softmax denominator rows land at psum base 0 —
            # custom-DVE ops ignore a nonzero source partition base, so the
            # reciprocal must read base 0), cols 64-127 = V dims
            vpo = vp_pool.tile([128, NKT, NH_LOC, 128], BF16)
            nc.vector.memset(vpo[:, :, :, 0:64], 1.0)

            agin = [[dram.tile([128, 512], BF16, name=f"agin{h}_{q}",
                               tag=f"agin{h}_{q}") for q in range(NQ4)]
                    for h in range(NHP)]
            agout = [[dram.tile([GROUP, 128, 512], BF16, name=f"agout{h}_{q}",
                                tag=f"agout{h}_{q}") for q in range(NQ4)]
                     for h in range(NHP)]

            # ---- phase 1: projections ----
            with (
                tc.tile_pool(name="xw", bufs=1) as xw_pool,
                tc.tile_pool(name="ppsum", bufs=1, space="PSUM") as ppsum,
            ):
                wq = xw_pool.tile([128, NKD, HFEAT], BF16)
                wk = xw_pool.tile([128, NKD, HFEAT], BF16)
                wv = xw_pool.tile([128, NKD, HFEAT], BF16)
                xt = xw_pool.tile([128, NKD, T], BF16)
                for k in range(NKD):
                    # xt/wk/wq feed the first matmul chains; wv is needed last
                    nc.sync.dma_start(
                        xt[:, k, :], xT.ap()[128 * k:128 * (k + 1), :])
                    for wtile, wdram in ((wk, wkT), (wq, wqT), (wv, wvT)):
                        nc.sync.dma_start(
                            wtile[:, k, :], wdram.ap()[128 * k:128 * (k + 1), :])

                # Q.T / K.T feature-major [256, T]; k-inner accumulation
                # chains hide LDWEIGHTS under the previous matmul (measured
                # 131 vs 260 ns/MM).  hp-0 halves first so attention can
                # start while hp-1 projects.
                ncopy = 0
                nacc = 0
                for m in range(2):
                    for wtile, dst in ((wk, kt), (wq, qt)):
                        for n in range(4):
                            acc = ppsum.tile([128, 512], F32,
                                             tag=f"acc{nacc % 8}")
                            nacc += 1
                            for k in range(NKD):
                                nc.tensor.matmul(
                                    acc[:],
                                    wtile[:, k, 128 * m:128 * (m + 1)],
                                    xt[:, k, 512 * n:512 * (n + 1)],
                                    start=(k == 0), stop=(k == NKD - 1),
                                )
                            ncopy += 1
                            if ncopy % 2 == 0:
                                nc.vector.tensor_copy(
                                    dst[m][:, 512 * n:512 * (n + 1)], acc[:])
                            else:
                                nc.scalar.copy(
                                    dst[m][:, 512 * n:512 * (n + 1)], acc[:])

                # V token-major -> vpo[:, mt, h, 0:64] (bf16)
                vcopy = None
                for mt in range(NKT):
                    ps = ppsum.tile([128, HFEAT], F32, tag=f"acc{mt % 8}")
                    for k in range(NKD):
                        nc.tensor.matmul(
                            ps[:],
                            xt[:, k, 128 * mt:128 * (mt + 1)],
                            wv[:, k, :],
                            start=(k == 0), stop=(k == NKD - 1),
                        )
                    src = ps[:].rearrange("p (h d) -> p h d", h=NH_LOC)
                    if mt % 2 == 0:
                        vcopy = nc.vector.tensor_copy(
                            vpo[:, mt, :, 64:128], src)
                    else:
                        vcopy = nc.scalar.copy(vpo[:, mt, :, 64:128], src)

            # ---- phase 2 + 3 share the wo weights ----
            with tc.tile_pool(name="wo", bufs=1) as wo_pool:
                wo = wo_pool.tile([128, NKD, KD], BF16)
                for k in range(NKD):
                    wdma = nc.sync.dma_start(
                        wo[:, k, :], woT.ap()[128 * k:128 * (k + 1), :])
                    tile.add_dep_helper(vcopy.ins, wdma.ins, sync=False,
                                        reason="defer wo prefetch past proj")

                with (
                    tc.tile_pool(name="spsum", bufs=2, space="PSUM") as spsum,
                    tc.tile_pool(name="opsum", bufs=1, space="PSUM") as opsum,
                    tc.tile_pool(name="ypsum", bufs=1, space="PSUM") as ypsum,
                    tc.tile_pool(name="orhs", bufs=2) as orhs_pool,
                    tc.tile_pool(name="yt", bufs=4) as yt_pool,
                ):
                    pid = nc.partition_id()
                    colo_sub = (pid & 3) * TSUB
                    rhs = [orhs_pool.tile([128, TSUB], BF16, name=f"rhs{k}",
                                          tag=f"rhs{k}") for k in range(NKD)]
                    # PSUM tiles are bank-granular: pack the 8 m-tiles into
                    # one [128, 8, 128] tile (4 KB/partition = 2 banks)
                    ypk = ypsum.tile([128, NKD, TSUB], F32, name="ypk",
                                     tag="ypk")
                    yps = [ypk[:, m, :] for m in range(NKD)]

                    y3a = [yt_pool.tile([128, TSUB], F32, name=f"y3a{m}",
                                        tag=f"y3a{m}") for m in range(NKD)]

                    def wo_rhs_dma(q4, hp):
                        for s in range(GROUP):
                            kidx = GROUP * hp + s
                            nc.sync.dma_start(
                                rhs[kidx][:],
                                agout[hp][q4][bass.ds(s, 1), :,
                                              bass.ds(colo_sub, TSUB)
                                              ].squeeze(0),
                            )

                    def wo_chunk(q4):
                        # this core's 128-token sub-slice of q-chunk q4;
                        # output copies interleave with the matmuls
                        wo_rhs_dma(q4, 0)
                        wo_rhs_dma(q4, 1)
                        for m in range(NKD):
                            for kidx in range(NKD):
                                nc.tensor.matmul(
                                    yps[m],
                                    wo[:, kidx, 128 * m:128 * (m + 1)],
                                    rhs[kidx][:],
                                    start=(kidx == 0),
                                    stop=(kidx == NKD - 1),
                                )
                            yt_s = yt_pool.tile([128, TSUB], F32, tag="yt")
                            if m % 2 == 0:
                                nc.vector.tensor_copy(yt_s[:], yps[m])
                            else:
                                nc.scalar.copy(yt_s[:], yps[m])
                            nc.sync.dma_start(
                                yT.ap()[128 * m:128 * (m + 1), q4, :],
                                yt_s[:])

                    def wo3_first_half():
                        # hp-0 half of the last q-chunk's Wo: closed psum
                        # group, bounced to SBUF; added back in the tail half
                        wo_rhs_dma(3, 0)
                        for m in range(NKD):
                            for kidx in range(GROUP):
                                nc.tensor.matmul(
                                    yps[m],
                                    wo[:, kidx, 128 * m:128 * (m + 1)],
                                    rhs[kidx][:],
                                    start=(kidx == 0),
                                    stop=(kidx == GROUP - 1),
                                )
                            if m % 2 == 0:
                                nc.vector.tensor_copy(y3a[m][:], yps[m])
                            else:
                                nc.scalar.copy(y3a[m][:], yps[m])

                    def wo3_second_half():
                        wo_rhs_dma(3, 1)
                        for m in range(NKD):
                            for s in range(GROUP):
                                kidx = GROUP + s
                                nc.tensor.matmul(
                                    yps[m],
                                    wo[:, kidx, 128 * m:128 * (m + 1)],
                                    rhs[kidx][:],
                                    start=(s == 0),
                                    stop=(s == GROUP - 1),
                                )
                            yt_s = yt_pool.tile([128, TSUB], F32, tag="yt")
                            nc.vector.tensor_add(yt_s[:], yps[m], y3a[m][:])
                            nc.sync.dma_start(
                                yT.ap()[128 * m:128 * (m + 1), 3, :],
                                yt_s[:])

                    # ---- phase 2: attention ----
                    # trailing od/normalize queue: od(k) is emitted two
                    # ktiles after exp(k) is issued, so the in-order PE queue
                    # never waits on the ~1.2us exp latency.
                    pending = []

                    def pop_pending(depth=2):
                        while len(pending) >= depth:
                            od, fin = pending.pop(0)
                            od()
                            if fin is not None:
                                fin()

                    for ci, (hp, q4) in enumerate(CHUNKS):
                        if ci in WO_EMIT:
                            pop_pending(1)
                            wo_chunk(WO_EMIT[ci])
                        hA, hB = 2 * hp, 2 * hp + 1
                        qs = slice(512 * q4, 512 * (q4 + 1))
                        # denominators rows 0-63, O.T rows 64-127;
                        # cols 0-511 head A, cols 512-1023 head B
                        opd = opsum.tile([128, 1024], F32, tag="opd")
                        pts = []

                        def odslot(km, opd=opd, pts=pts, hA=hA, hB=hB):
                            nc.tensor.matmul(
                                opd[:, 0:512], vpo[:, km, hA, :],
                                pts[km][:, 0:512],
                                start=(km == 0), stop=(km == NKT - 1))
                            nc.tensor.matmul(
                                opd[:, 512:1024], vpo[:, km, hB, :],
                                pts[km][:, 512:1024],
                                start=(km == 0), stop=(km == NKT - 1))

                        def finalize(opd=opd, hp=hp, q4=q4):
                            rb = rb_pool.tile([64, 1024], F32, tag="rb")
                            nc.vector.reciprocal_approx_fast(
                                rb[:], opd[0:64, :])
                            onorm = on_pool.tile([64, 1024], BF16, tag="on")
                            nc.vector.tensor_mul(onorm[:], opd[64:128, :], rb[:])
                            nc.sync.dma_start(
                                agin[hp][q4][0:64, :], onorm[:, 0:512])
                            nc.sync.dma_start(
                                agin[hp][q4][64:128, :], onorm[:, 512:1024])
                            nc.gpsimd.collective_compute(
                                "AllGather",
                                mybir.AluOpType.bypass,
                                replica_groups=rgroups,
                                ins=[agin[hp][q4].opt()],
                                outs=[agout[hp][q4].opt()],
                            )

                        for ktile in range(NKT):
                            ks = slice(128 * ktile, 128 * (ktile + 1))
                            sp = spsum.tile([128, 1024], F32, tag="sp")
                            nc.tensor.matmul(
                                sp[:, 0:512], kt[hp][0:64, ks],
                                qt[hp][0:64, qs], start=True, stop=True)
                            nc.tensor.matmul(
                                sp[:, 512:1024], kt[hp][64:128, ks],
                                qt[hp][64:128, qs], start=True, stop=True)
                            pt = pt_pool.tile([128, 1024], BF16, tag="pt")
                            if ktile in DVE_KT:
                                nc.vector._custom_dve(
                                    exp_op, out=pt[:], in0=sp[:],
                                    s0=EXP_C0, s1=EXP_C1, imm2=EXP_C2)
                            else:
                                nc.scalar.activation(pt[:], sp[:], AF.Exp,
                                                     scale=0.03125)
                            pts.append(pt)
                            pop_pending(3)
                            if ci == len(CHUNKS) - 1 and ktile == 12:
                                # hp-0 half of the last q-chunk's Wo runs
                                # while the final attention chunk computes
                                wo3_first_half()
                            pending.append(
                                ((lambda km=ktile, f=odslot: f(km)),
                                 finalize if ktile == NKT - 1 else None))
                    pop_pending(1)
                    wo3_second_half()

    nc.compile()
    return nc


def _get_nc():
    if "nc" not in _CACHE:
        _CACHE["nc"] = _build()
    return _CACHE["nc"]


def _make_in_maps(x, Wq, Wk, Wv, Wo):
    # Wo rows permuted to match the AllGather assembly order:
    # f = 128*kidx + row, kidx = 4*hp + s  ->  head 4s+2hp+(row>=64), dim row%64
    f = np.arange(KD)
    kidx, row = f // 128, f % 128
    hp, s = kidx // GROUP, kidx % GROUP
    head = GROUP * s + 2 * hp + (row >= HS)
    perm = head * HS + row % HS
    woTp = np.ascontiguousarray(Wo.T[perm]).astype(ml_dtypes.bfloat16)

    in_maps = []
    for c in range(NCORES):
        g, r = c // GROUP, c % GROUP
        rows = slice(r * HFEAT, (r + 1) * HFEAT)
        in_maps.append({
            "xT": np.ascontiguousarray(x[g].T).astype(ml_dtypes.bfloat16),
            "wqT": np.ascontiguousarray(Wq[rows].T).astype(ml_dtypes.bfloat16),
            "wkT": np.ascontiguousarray(Wk[rows].T).astype(ml_dtypes.bfloat16),
            "wvT": np.ascontiguousarray(Wv[rows].T).astype(ml_dtypes.bfloat16),
            "woT": woTp,
        })
    return in_maps


def kernel(x, Wq, Wk, Wv, Wo):
    from concourse import bass_utils

    x = np.asarray(x, dtype=np.float32)
    Wq = np.asarray(Wq, dtype=np.float32)
    Wk = np.asarray(Wk, dtype=np.float32)
    Wv = np.asarray(Wv, dtype=np.float32)
    Wo = np.asarray(Wo, dtype=np.float32)

    nc = _get_nc()
    in_maps = _make_in_maps(x, Wq, Wk, Wv, Wo)
    res = bass_utils.run_bass_kernel_spmd(nc, in_maps, core_ids=list(range(NCORES)))

    out = np.empty((B, T, KD), dtype=np.float32)
    for c in range(NCORES):
        g, r = c // GROUP, c % GROUP
        yTc = res.results[c]["yT"]          # [KD, NQ4, TSUB]
        for q4 in range(NQ4):
            out[g, 512 * q4 + TSUB * r: 512 * q4 + TSUB * (r + 1), :] = \
                yTc[:, q4, :].T
    return out
